# revision 2
# baseline (speedup 1.0000x reference)
"""DbrxAttention (GQA + RoPE + causal) on 8 Trainium2 NeuronCores.

Tensor-parallel over heads: core c owns q heads [6c, 6c+6) and kv head c.
Per core: QKV projection (transposed layout, bf16, weight-shared matmul
pairs, per-chunk RoPE/v-transpose fused into the projection pipeline),
causal attention (scores transposed: kv on partitions, q on free dim;
softmax denominator accumulated on vector+gpsimd engines, one small
matmul per chunk), 3-way split AllToAll (per head-pair, overlapped with
attention), then each core computes a 256-row sequence block of the
output projection against the full w_out, with the first blocks'
contraction split so work for already-arrived head groups can run while
the last AllToAll is still in flight.

kernel(**inputs) takes the full unsharded inputs and returns the full output.
"""

import math

import numpy as np
import ml_dtypes

import concourse.bass as bass
import concourse.mybir as mybir
from concourse import bacc
import concourse.tile as tile
from concourse.bass_utils import run_bass_kernel_spmd
from concourse.masks import make_identity

BF16 = mybir.dt.bfloat16
F32 = mybir.dt.float32
NP_BF16 = ml_dtypes.bfloat16

# full-size problem config
B, S, D = 1, 2048, 6144
H, KV, HD = 48, 8, 128
R = 8  # cores


class Cfg:
    def __init__(self, S=2048, KO=48, NQ=6, R=8, DO=6144, IC=512, CH=512,
                 OT=512, GH=2, KA=32, NSPLIT=4):
        self.S = S          # sequence length
        self.KO = KO        # contraction k-tiles for QKV (D = KO*128)
        self.NQ = NQ        # q heads per core
        self.R = R          # cores
        self.DO = DO        # out_proj output dim
        self.IC = IC        # attention i-chunk (free dim per scores matmul)
        self.CH = CH        # QKV s-chunk (pair of CH/2 matmuls)
        self.OT = OT        # out_proj n-chunk
        self.GH = GH        # heads per AllToAll group
        self.KA = KA        # out_proj k-tiles available before last A2A
        self.NSPLIT = NSPLIT  # nt blocks using split-k accumulation
        self.D = KO * 128
        self.SB = S // R    # seq block per core after AllToAll
        self.KO2 = R * NQ   # contraction k-tiles for out_proj (H*HD = KO2*128)
        self.NT = DO // OT
        self.NG = NQ // GH  # number of A2A groups
        assert S % R == 0 and S % IC == 0 and S % CH == 0 and DO % OT == 0
        assert IC % 128 == 0
        assert NQ % GH == 0
        assert KA == self.KO2 - self.R * GH  # kos left for the last group


def build(cfg: Cfg) -> bacc.Bacc:
    S, KO, NQ, IC, CH = cfg.S, cfg.KO, cfg.NQ, cfg.IC, cfg.CH
    NE = NQ + 2            # qkv e-tiles per core
    NEP = NE // 2          # e-tile pairs (wq DMA chunks)
    NCH = S // CH
    NIC = S // IC
    ND = IC // 128         # diagonal j-tiles per i-chunk
    NJ = S // 128
    SB, KO2, NT, DO, OT = cfg.SB, cfg.KO2, cfg.NT, cfg.DO, cfg.OT
    GH, NG, KA, NSPLIT = cfg.GH, cfg.NG, cfg.KA, cfg.NSPLIT
    CHH = CH // 2
    softmax_scale = 1.0 / math.sqrt(HD)
    ET_K, ET_V = 0, 1      # e-tile indices of k and v
    MULT = mybir.AluOpType.mult

    nc = bacc.Bacc("TRN2", target_bir_lowering=False, debug=False,
                   num_devices=cfg.R)

    hid_d = nc.dram_tensor("hid", [NCH, 128, KO, CH], BF16,
                           kind="ExternalInput")
    wq_d = nc.dram_tensor("wq", [NEP, 128, KO, 256], BF16,
                          kind="ExternalInput")
    wo_d = nc.dram_tensor("wo", [NT, 128, KO2, OT], BF16,
                          kind="ExternalInput")
    cos_d = nc.dram_tensor("cosT", [128, S], BF16, kind="ExternalInput")
    sin_d = nc.dram_tensor("sinT", [128, S], BF16, kind="ExternalInput")
    msk_d = nc.dram_tensor("masks", [128, ND, IC], BF16, kind="ExternalInput")
    out_d = nc.dram_tensor("out", [SB, DO], F32, kind="ExternalOutput")

    with (
        tile.TileContext(nc) as tc,
        tc.tile_pool(name="dram", bufs=1, space="DRAM") as dram,
    ):
        a2a_in = [dram.tile([cfg.R, GH * 128, SB], BF16,
                            name=f"a2a_in{g}") for g in range(NG)]
        a2a_out = [dram.tile([cfg.R, GH * 128, SB], BF16,
                             name=f"a2a_out{g}") for g in range(NG)]

        with (
            tc.tile_pool(name="big", bufs=1) as big,
            tc.tile_pool(name="psk", bufs=3, space="PSUM") as psk,
            tc.tile_pool(name="psv", bufs=2, space="PSUM") as psv,
        ):
            qkv_sb = big.tile([128, NE, S], BF16)
            ones_sb = big.tile([128, 1], BF16)
            nc.gpsimd.memset(ones_sb[:], 1.0)
            v_nat = big.tile([128, NJ, 128], BF16)

            # ---- phase 1: QKV projection (transposed: [e, s]) with fused
            # per-chunk RoPE and v-transpose ----
            with (
                tc.tile_pool(name="ropec", bufs=1) as ropec,
                tc.tile_pool(name="wqp", bufs=2) as wqp,
                tc.tile_pool(name="hidp", bufs=2) as hidp,
                tc.tile_pool(name="ropep", bufs=2) as ropep,
            ):
                cos_sb = ropec.tile([128, S], BF16)
                nc.sync.dma_start(cos_sb[:], cos_d.ap())
                sin_sb = ropec.tile([128, S], BF16)
                nc.sync.dma_start(sin_sb[:], sin_d.ap())
                ident = ropec.tile([128, 128], BF16)
                make_identity(nc, ident[:])

                def rope_chunk(et, ci):
                    sl = slice(ci * CH, (ci + 1) * CH)
                    x = qkv_sb[:, et, sl]
                    rot = ropep.tile([128, CH], BF16, tag="rot",
                                     name=f"rot{et}_{ci}")
                    nc.scalar.copy(rot[0:64, :], x[64:128, :])
                    nc.scalar.copy(rot[64:128, :], x[0:64, :])
                    nc.vector.tensor_mul(rot[:], rot[:], sin_sb[:, sl])
                    nc.vector.tensor_mul(x, x, cos_sb[:, sl])
                    nc.vector.tensor_add(x, x, rot[:])

                def vT_chunk(ci):
                    for st in range(ci * (CH // 128), (ci + 1) * (CH // 128)):
                        pt = psv.tile([128, 128], BF16, tag="pv",
                                      name=f"tp{st}")
                        nc.tensor.transpose(
                            pt[:], qkv_sb[:, ET_V, st * 128:(st + 1) * 128],
                            ident[:])
                        nc.vector.tensor_copy(v_nat[:, st, :], pt[:])

                for half in range(2):
                    wq_t = []
                    for epl in range(2):
                        ep = half * 2 + epl
                        w = wqp.tile([128, KO, 256], BF16, tag="wq",
                                     name=f"wq{ep}")
                        nc.sync.dma_start(w[:], wq_d.ap()[ep])
                        wq_t.append(w)
                    for ci in range(NCH):
                        hid_t = hidp.tile([128, KO, CH], BF16, tag="hid",
                                          name=f"hid{half}_{ci}")
                        nc.sync.dma_start(hid_t[:], hid_d.ap()[ci])
                        for epl in range(2):
                            for el in range(2):
                                et = half * 4 + epl * 2 + el
                                ps = psk.tile([128, 1024], F32, tag="ps",
                                              name=f"qk{et}_{ci}")
                                for ko in range(KO):
                                    w = wq_t[epl][:, ko,
                                                  el * 128:(el + 1) * 128]
                                    nc.tensor.matmul(
                                        ps[:, :CHH], lhsT=w,
                                        rhs=hid_t[:, ko, :CHH],
                                        start=(ko == 0), stop=(ko == KO - 1))
                                    nc.tensor.matmul(
                                        ps[:, 512:512 + CHH], lhsT=w,
                                        rhs=hid_t[:, ko, CHH:],
                                        start=(ko == 0), stop=(ko == KO - 1))
                                nc.vector.tensor_copy(
                                    qkv_sb[:, et, ci * CH:ci * CH + CHH],
                                    ps[:, :CHH])
                                nc.vector.tensor_copy(
                                    qkv_sb[:, et,
                                           ci * CH + CHH:(ci + 1) * CH],
                                    ps[:, 512:512 + CHH])
                                # fused epilogue for this (e-tile, chunk)
                                if et == ET_V:
                                    vT_chunk(ci)
                                else:
                                    rope_chunk(et, ci)

            # ---- phase 2+3: attention, normalize per chunk, split A2A ----
            with (
                tc.tile_pool(name="attw", bufs=1) as attw,
                tc.tile_pool(name="pp", bufs=4) as pp,
                tc.tile_pool(name="accp", bufs=2) as accp,
                tc.tile_pool(name="rp", bufs=2) as rp,
                tc.tile_pool(name="rbp", bufs=2) as rbp,
                tc.tile_pool(name="stg", bufs=3) as stg,
            ):
                msk_sb = attw.tile([128, ND, IC], BF16)
                nc.sync.dma_start(msk_sb[:], msk_d.ap())
                kT = qkv_sb[:, ET_K, :]
                for h in range(NQ):
                    qT = qkv_sb[:, 2 + h, :]
                    g, hl = divmod(h, GH)
                    for ci in range(NIC):
                        jt_max = (ci + 1) * ND
                        pv = psv.tile([128, IC], F32, tag="pv",
                                      name=f"pv{h}_{ci}")
                        acc0 = accp.tile([128, IC], F32, tag="a0",
                                         name=f"a0_{h}_{ci}")
                        acc1 = accp.tile([128, IC], F32, tag="a1",
                                         name=f"a1_{h}_{ci}")
                        for jp in range(jt_max // 2):
                            sc = psk.tile([128, 1024], F32, tag="ps",
                                          name=f"sc{h}_{ci}_{jp}")
                            for u in range(2):
                                jt = 2 * jp + u
                                nc.tensor.matmul(
                                    sc[:, u * 512:u * 512 + IC],
                                    lhsT=kT[:, jt * 128:(jt + 1) * 128],
                                    rhs=qT[:, ci * IC:(ci + 1) * IC],
                                    start=True, stop=True)
                            p2 = pp.tile([128, 1024], BF16, tag="p",
                                         name=f"p{h}_{ci}_{jp}")
                            nc.scalar.activation(
                                p2[:], sc[:],
                                mybir.ActivationFunctionType.Exp,
                                scale=softmax_scale)
                            for u in range(2):
                                jt = 2 * jp + u
                                pu = p2[:, u * 512:u * 512 + IC]
                                if jt >= ci * ND:
                                    nc.vector.tensor_mul(
                                        pu, pu, msk_sb[:, jt - ci * ND, :])
                                nc.tensor.matmul(
                                    pv[:], lhsT=v_nat[:, jt, :], rhs=pu,
                                    start=(jt == 0), stop=(jt == jt_max - 1))
                                eng = nc.gpsimd if u == 0 else nc.vector
                                acc = acc0 if u == 0 else acc1
                                if jp == 0:
                                    eng.tensor_copy(acc[:], pu)
                                else:
                                    eng.tensor_add(acc[:], acc[:], pu)
                        # chunk epilogue: denominator, normalize, ship
                        accb = rbp.tile([128, IC], BF16, tag="ab",
                                        name=f"ab{h}_{ci}")
                        nc.vector.tensor_add(accb[:], acc0[:], acc1[:])
                        dq = psv.tile([1, IC], F32, tag="pv",
                                      name=f"dq{h}_{ci}")
                        nc.tensor.matmul(dq[:], lhsT=ones_sb[:, 0:1],
                                         rhs=accb[:], start=True, stop=True)
                        r_sb = rp.tile([1, IC], F32, tag="r",
                                       name=f"r{h}_{ci}")
                        nc.vector.reciprocal_approx_fast(r_sb[:], dq[:])
                        rb = rbp.tile([128, IC], F32, tag="rb",
                                      name=f"rb{h}_{ci}")
                        nc.gpsimd.partition_broadcast(rb[:], r_sb[:])
                        o = stg.tile([128, IC], BF16, tag="o",
                                     name=f"o{h}_{ci}")
                        nc.vector.scalar_tensor_tensor(
                            o[:], pv[:], 1.0, rb[:], MULT, MULT)
                        nc.sync.dma_start(
                            a2a_in[g][2 * ci:2 * ci + 2,
                                      hl * 128:(hl + 1) * 128, :]
                            .rearrange("r p s -> p r s"),
                            o[:].rearrange("p (r s) -> p r s", r=2))
                    if hl == GH - 1:
                        nc.gpsimd.collective_compute(
                            "AllToAll", mybir.AluOpType.bypass,
                            replica_groups=[list(range(cfg.R))],
                            ins=[a2a_in[g][:]], outs=[a2a_out[g][:]])

        # ---- phase 4: out_proj on this core's seq block ----
        KB = KO2 - KA
        with (
            tc.tile_pool(name="otf", bufs=1) as otf,
            tc.tile_pool(name="wopA", bufs=4) as wopA,
            tc.tile_pool(name="wopB", bufs=2) as wopB,
            tc.tile_pool(name="obp", bufs=2) as obp,
            tc.tile_pool(name="psB", bufs=8, space="PSUM") as psB,
        ):
            oT_full = otf.tile([128, KO2, SB], BF16)
            for g in range(NG):
                nc.sync.dma_start(
                    oT_full[:, g * (cfg.R * GH):(g + 1) * (cfg.R * GH), :]
                    .rearrange("p (r hl) s -> p r hl s", r=cfg.R),
                    a2a_out[g][:].rearrange("r (hl p) s -> p r hl s", hl=GH))

            def emit_block(ps, wo_t, mi, ko0, nko, start, stop):
                for k in range(nko):
                    ko = ko0 + k
                    nc.tensor.matmul(
                        ps[:], lhsT=oT_full[:, ko, mi * 128:(mi + 1) * 128],
                        rhs=wo_t[:, k, :],
                        start=(start and k == 0),
                        stop=(stop and k == nko - 1))

            def finish_block(ps, nt, mi):
                ob = obp.tile([128, OT], F32, tag="ob", name=f"ob{nt}_{mi}")
                nc.vector.tensor_copy(ob[:], ps[:])
                nc.sync.dma_start(
                    out_d.ap()[mi * 128:(mi + 1) * 128,
                               nt * OT:(nt + 1) * OT],
                    ob[:])

            split_ps = {}
            for nt in range(NSPLIT):
                woA = wopA.tile([128, KA, OT], BF16, tag="woA",
                                name=f"woA{nt}")
                nc.sync.dma_start(woA[:], wo_d.ap()[nt][:, :KA, :])
                for mi in range(SB // 128):
                    ps = psB.tile([128, OT], F32, tag="po",
                                  name=f"po{nt}_{mi}")
                    emit_block(ps, woA, mi, 0, KA, True, False)
                    split_ps[(nt, mi)] = ps
            for nt in range(NSPLIT):
                woB = wopB.tile([128, KB, OT], BF16, tag="woB",
                                name=f"woB{nt}")
                nc.sync.dma_start(woB[:], wo_d.ap()[nt][:, KA:, :])
                for mi in range(SB // 128):
                    ps = split_ps[(nt, mi)]
                    emit_block(ps, woB, mi, KA, KB, False, True)
                    finish_block(ps, nt, mi)
            for nt in range(NSPLIT, NT):
                woA = wopA.tile([128, KA, OT], BF16, tag="woA",
                                name=f"woA{nt}")
                nc.sync.dma_start(woA[:], wo_d.ap()[nt][:, :KA, :])
                woB = wopB.tile([128, KB, OT], BF16, tag="woB",
                                name=f"woB{nt}")
                nc.sync.dma_start(woB[:], wo_d.ap()[nt][:, KA:, :])
                for mi in range(SB // 128):
                    ps = psB.tile([128, OT], F32, tag="po",
                                  name=f"po{nt}_{mi}")
                    emit_block(ps, woA, mi, 0, KA, True, False)
                    emit_block(ps, woB, mi, KA, KB, False, True)
                    finish_block(ps, nt, mi)

    nc.compile()
    return nc


def make_masks(cfg: Cfg) -> np.ndarray:
    ND = cfg.IC // 128
    jj = np.arange(128)[:, None, None]
    rr = np.arange(ND)[None, :, None]
    ii = np.arange(cfg.IC)[None, None, :]
    return (jj + 128 * rr <= ii).astype(NP_BF16)


def shard_inputs(cfg: Cfg, hidden_states, cos, sin, w_qkv, w_out,
                 n_heads, n_kv):
    """Build per-core input maps (host-side shard + bf16 cast + layout)."""
    S, KO, NQ, R = cfg.S, cfg.KO, cfg.NQ, cfg.R
    D = cfg.D
    NCH, CH = S // cfg.CH, cfg.CH
    hid_T = np.ascontiguousarray(hidden_states.reshape(S, D).T)  # [D, S]
    # [NCH, 128, KO, CH]
    hid_l = (hid_T.reshape(KO, 128, NCH, CH).transpose(2, 1, 0, 3)
             .astype(NP_BF16))
    hid_l = np.ascontiguousarray(hid_l)
    NT, OT, KO2 = cfg.NT, cfg.OT, cfg.KO2
    # reorder w_out rows so contraction tile ko2 = g*(R*GH) + r*GH + hl
    # maps to global head r*NQ + g*GH + hl
    NG, GH = cfg.NG, cfg.GH
    w_re = (w_out.reshape(R, NG, GH, 128, cfg.DO).transpose(1, 0, 2, 3, 4)
            .reshape(KO2 * 128, cfg.DO))
    wo_l = (w_re.reshape(KO2, 128, NT, OT).transpose(2, 1, 0, 3)
            .astype(NP_BF16))
    wo_l = np.ascontiguousarray(wo_l)
    cos_T = cos.T.astype(NP_BF16)  # [HD, S]
    sin_T = sin.T
    sinS = np.concatenate([-sin_T[:64], sin_T[64:]], axis=0).astype(NP_BF16)
    masks = make_masks(cfg)

    in_maps = []
    NE = NQ + 2
    NEP = NE // 2
    for c in range(R):
        qs = c * NQ * 128
        # e-tile order: k, v, q0..q5
        wsh = np.concatenate([
            w_qkv[:, n_heads * HD + c * 128: n_heads * HD + (c + 1) * 128],
            w_qkv[:, (n_heads + n_kv) * HD + c * 128:
                  (n_heads + n_kv) * HD + (c + 1) * 128],
            w_qkv[:, qs:qs + NQ * 128],
        ], axis=1)  # [D, NE*128]
        wq_l = (wsh.reshape(KO, 128, NEP, 256).transpose(2, 1, 0, 3)
                .astype(NP_BF16))
        in_maps.append({
            "hid": hid_l, "wq": np.ascontiguousarray(wq_l), "wo": wo_l,
            "cosT": cos_T, "sinT": sinS, "masks": masks,
        })
    return in_maps


_cached = {}


def _get_nc(cfg: Cfg):
    key = tuple(sorted(cfg.__dict__.items()))
    if key not in _cached:
        _cached[key] = build(cfg)
    return _cached[key]


def run(cfg: Cfg, in_maps, **kwargs):
    nc = _get_nc(cfg)
    res = run_bass_kernel_spmd(nc, in_maps, core_ids=list(range(cfg.R)),
                               **kwargs)
    out = np.concatenate([res.results[c]["out"] for c in range(cfg.R)],
                         axis=0)
    return out, res


def kernel(hidden_states, cos, sin, w_qkv, w_out):
    cfg = Cfg()
    hidden_states = np.asarray(hidden_states, dtype=np.float32)
    cos = np.asarray(cos, dtype=np.float32)
    sin = np.asarray(sin, dtype=np.float32)
    w_qkv = np.asarray(w_qkv, dtype=np.float32)
    w_out = np.asarray(w_out, dtype=np.float32)
    in_maps = shard_inputs(cfg, hidden_states, cos, sin, w_qkv, w_out, H, KV)
    out, _ = run(cfg, in_maps)
    return out.reshape(B, S, D).astype(np.float32)


# revision 5
# speedup vs baseline: 1.2668x; 1.2668x over previous
"""DbrxAttention (GQA + RoPE + causal) on 8 Trainium2 NeuronCores.

Tensor-parallel over heads: core c owns q heads [6c, 6c+6) and kv head c.
Per core: QKV projection (transposed layout, bf16, weight-shared matmul
pairs, per-chunk RoPE/v-transpose fused into the projection pipeline),
causal attention (scores transposed: kv on partitions, q on free dim;
softmax denominator accumulated on vector+gpsimd engines, one small
matmul per chunk), 3-way split AllToAll (per head-pair, overlapped with
attention), then each core computes a 256-row sequence block of the
output projection against the full w_out, with the first blocks'
contraction split so work for already-arrived head groups can run while
the last AllToAll is still in flight.

kernel(**inputs) takes the full unsharded inputs and returns the full output.
"""

import math

import numpy as np
import ml_dtypes

import concourse.bass as bass
import concourse.mybir as mybir
from concourse import bacc
import concourse.tile as tile
from concourse.bass_utils import run_bass_kernel_spmd
from concourse.masks import make_identity

BF16 = mybir.dt.bfloat16
F32 = mybir.dt.float32
NP_BF16 = ml_dtypes.bfloat16

# full-size problem config
B, S, D = 1, 2048, 6144
H, KV, HD = 48, 8, 128
R = 8  # cores


class Cfg:
    def __init__(self, S=2048, KO=48, NQ=6, R=8, DO=6144, IC=512, CH=512,
                 OT=512, GH=2, KA=32, NSPLIT=4):
        self.S = S          # sequence length
        self.KO = KO        # contraction k-tiles for QKV (D = KO*128)
        self.NQ = NQ        # q heads per core
        self.R = R          # cores
        self.DO = DO        # out_proj output dim
        self.IC = IC        # attention i-chunk (free dim per scores matmul)
        self.CH = CH        # QKV s-chunk (pair of CH/2 matmuls)
        self.OT = OT        # out_proj n-chunk
        self.GH = GH        # heads per AllToAll group
        self.KA = KA        # out_proj k-tiles available before last A2A
        self.NSPLIT = NSPLIT  # nt blocks using split-k accumulation
        self.D = KO * 128
        self.SB = S // R    # seq block per core after AllToAll
        self.KO2 = R * NQ   # contraction k-tiles for out_proj (H*HD = KO2*128)
        self.NT = DO // OT
        self.NG = NQ // GH  # number of A2A groups
        assert S % R == 0 and S % IC == 0 and S % CH == 0 and DO % OT == 0
        assert IC % 128 == 0
        assert NQ % GH == 0
        assert KA == self.KO2 - self.R * GH  # kos left for the last group


def build(cfg: Cfg) -> bacc.Bacc:
    S, KO, NQ, IC, CH = cfg.S, cfg.KO, cfg.NQ, cfg.IC, cfg.CH
    NE = NQ + 2            # qkv e-tiles per core
    NEP = NE // 2          # e-tile pairs (wq DMA chunks)
    NCH = S // CH
    NIC = S // IC
    ND = IC // 128         # diagonal j-tiles per i-chunk
    NJ = S // 128
    SB, KO2, NT, DO, OT = cfg.SB, cfg.KO2, cfg.NT, cfg.DO, cfg.OT
    GH, NG, KA, NSPLIT = cfg.GH, cfg.NG, cfg.KA, cfg.NSPLIT
    CHH = CH // 2
    softmax_scale = 1.0 / math.sqrt(HD)
    ET_K, ET_V = 0, 1      # e-tile indices of k and v
    MULT = mybir.AluOpType.mult

    nc = bacc.Bacc("TRN2", target_bir_lowering=False, debug=False,
                   num_devices=cfg.R)

    hid_d = nc.dram_tensor("hid", [NCH, 128, KO, CH], BF16,
                           kind="ExternalInput")
    wq_d = nc.dram_tensor("wq", [NEP, 128, KO, 256], BF16,
                          kind="ExternalInput")
    wo_d = nc.dram_tensor("wo", [NT, 128, KO2, OT], BF16,
                          kind="ExternalInput")
    cos_d = nc.dram_tensor("cosT", [128, S], BF16, kind="ExternalInput")
    sin_d = nc.dram_tensor("sinT", [128, S], BF16, kind="ExternalInput")
    msk_d = nc.dram_tensor("masks", [128, ND, IC], BF16, kind="ExternalInput")
    out_d = nc.dram_tensor("out", [SB, DO], F32, kind="ExternalOutput")

    with (
        tile.TileContext(nc) as tc,
        tc.tile_pool(name="dram", bufs=1, space="DRAM") as dram,
    ):
        a2a_in = [dram.tile([cfg.R, GH * 128, SB], BF16,
                            name=f"a2a_in{g}") for g in range(NG)]
        a2a_out = [dram.tile([cfg.R, GH * 128, SB], BF16,
                             name=f"a2a_out{g}") for g in range(NG)]

        with (
            tc.tile_pool(name="big", bufs=1) as big,
            tc.tile_pool(name="psk", bufs=3, space="PSUM") as psk,
            tc.tile_pool(name="psv", bufs=1, space="PSUM") as psv,
            tc.tile_pool(name="psd", bufs=1, space="PSUM") as psd,
        ):
            qkv_sb = big.tile([128, NE, S], BF16)
            ones_sb = big.tile([128, 1], BF16)
            nc.gpsimd.memset(ones_sb[:], 1.0)
            v_nat = big.tile([128, NJ, 128], BF16)

            # ---- phase 1: QKV projection (transposed: [e, s]) with fused
            # per-chunk RoPE and v-transpose ----
            with (
                tc.tile_pool(name="ropec", bufs=1) as ropec,
                tc.tile_pool(name="wqp", bufs=2) as wqp,
                tc.tile_pool(name="hidp", bufs=2) as hidp,
                tc.tile_pool(name="ropep", bufs=2) as ropep,
            ):
                cos_sb = ropec.tile([128, S], BF16)
                nc.sync.dma_start(cos_sb[:], cos_d.ap())
                sin_sb = ropec.tile([128, S], BF16)
                nc.sync.dma_start(sin_sb[:], sin_d.ap())
                ident = ropec.tile([128, 128], BF16)
                make_identity(nc, ident[:])

                def rope_chunk(et, ci):
                    sl = slice(ci * CH, (ci + 1) * CH)
                    x = qkv_sb[:, et, sl]
                    rot = ropep.tile([128, CH], BF16, tag="rot",
                                     name=f"rot{et}_{ci}")
                    nc.scalar.copy(rot[0:64, :], x[64:128, :])
                    nc.scalar.copy(rot[64:128, :], x[0:64, :])
                    nc.vector.tensor_mul(rot[:], rot[:], sin_sb[:, sl])
                    nc.vector.tensor_mul(x, x, cos_sb[:, sl])
                    nc.vector.tensor_add(x, x, rot[:])

                def vT_chunk(ci):
                    for st in range(ci * (CH // 128), (ci + 1) * (CH // 128)):
                        pt = psv.tile([128, 128], BF16, tag="pv",
                                      name=f"tp{st}")
                        nc.tensor.transpose(
                            pt[:], qkv_sb[:, ET_V, st * 128:(st + 1) * 128],
                            ident[:])
                        nc.vector.tensor_copy(v_nat[:, st, :], pt[:])

                KOH = KO // 2
                for half in range(2):
                    wq_t = []
                    for epl in range(2):
                        ep = half * 2 + epl
                        wa = wqp.tile([128, KOH, 256], BF16, tag="wqA",
                                      name=f"wqA{ep}")
                        nc.sync.dma_start(wa[:], wq_d.ap()[ep][:, :KOH, :])
                        wb = wqp.tile([128, KOH, 256], BF16, tag="wqB",
                                      name=f"wqB{ep}")
                        nc.sync.dma_start(wb[:], wq_d.ap()[ep][:, KOH:, :])
                        wq_t.append((wa, wb))
                    for ci in range(NCH):
                        ha = hidp.tile([128, KOH, CH], BF16, tag="hidA",
                                       name=f"hidA{half}_{ci}")
                        nc.sync.dma_start(ha[:], hid_d.ap()[ci][:, :KOH, :])
                        hb = hidp.tile([128, KOH, CH], BF16, tag="hidB",
                                       name=f"hidB{half}_{ci}")
                        nc.sync.dma_start(hb[:], hid_d.ap()[ci][:, KOH:, :])
                        hid_t = (ha, hb)
                        for epl in range(2):
                            for el in range(2):
                                et = half * 4 + epl * 2 + el
                                ps = psk.tile([128, 1024], F32, tag="ps",
                                              name=f"qk{et}_{ci}")
                                for ko in range(KO):
                                    kh, kl = divmod(ko, KOH)
                                    w = wq_t[epl][kh][:, kl,
                                                      el * 128:(el + 1) * 128]
                                    h_t = hid_t[kh]
                                    nc.tensor.matmul(
                                        ps[:, :CHH], lhsT=w,
                                        rhs=h_t[:, kl, :CHH],
                                        start=(ko == 0), stop=(ko == KO - 1))
                                    nc.tensor.matmul(
                                        ps[:, 512:512 + CHH], lhsT=w,
                                        rhs=h_t[:, kl, CHH:],
                                        start=(ko == 0), stop=(ko == KO - 1))
                                nc.vector.tensor_copy(
                                    qkv_sb[:, et, ci * CH:ci * CH + CHH],
                                    ps[:, :CHH])
                                nc.vector.tensor_copy(
                                    qkv_sb[:, et,
                                           ci * CH + CHH:(ci + 1) * CH],
                                    ps[:, 512:512 + CHH])
                                # fused epilogue for this (e-tile, chunk)
                                if et == ET_V:
                                    vT_chunk(ci)
                                else:
                                    rope_chunk(et, ci)

            # ---- phase 2+3: attention, normalize per chunk, split A2A ----
            with (
                tc.tile_pool(name="attw", bufs=1) as attw,
                tc.tile_pool(name="pp", bufs=4) as pp,
                tc.tile_pool(name="rp", bufs=2) as rp,
                tc.tile_pool(name="rbp", bufs=2) as rbp,
                tc.tile_pool(name="stg", bufs=3) as stg,
            ):
                msk_sb = attw.tile([128, ND, IC], BF16)
                nc.sync.dma_start(msk_sb[:], msk_d.ap())
                kT = qkv_sb[:, ET_K, :]
                for h in range(NQ):
                    qT = qkv_sb[:, 2 + h, :]
                    g, hl = divmod(h, GH)
                    for ci in range(NIC):
                        jt_max = (ci + 1) * ND
                        pv = psv.tile([128, IC], F32, tag="pv",
                                      name=f"pv{h}_{ci}")
                        dq = psd.tile([1, IC], F32, tag="dq",
                                      name=f"dq{h}_{ci}")
                        for jp in range(jt_max // 2):
                            sc = psk.tile([128, 1024], F32, tag="ps",
                                          name=f"sc{h}_{ci}_{jp}")
                            for u in range(2):
                                jt = 2 * jp + u
                                nc.tensor.matmul(
                                    sc[:, u * 512:u * 512 + IC],
                                    lhsT=kT[:, jt * 128:(jt + 1) * 128],
                                    rhs=qT[:, ci * IC:(ci + 1) * IC],
                                    start=True, stop=True)
                            p2 = pp.tile([128, 1024], BF16, tag="p",
                                         name=f"p{h}_{ci}_{jp}")
                            nc.scalar.activation(
                                p2[:], sc[:],
                                mybir.ActivationFunctionType.Exp,
                                scale=softmax_scale)
                            for u in range(2):
                                jt = 2 * jp + u
                                pu = p2[:, u * 512:u * 512 + IC]
                                if jt >= ci * ND:
                                    nc.vector.tensor_mul(
                                        pu, pu, msk_sb[:, jt - ci * ND, :])
                                nc.tensor.matmul(
                                    pv[:], lhsT=v_nat[:, jt, :], rhs=pu,
                                    start=(jt == 0), stop=(jt == jt_max - 1))
                                nc.tensor.matmul(
                                    dq[:], lhsT=ones_sb[:, 0:1], rhs=pu,
                                    start=(jt == 0), stop=(jt == jt_max - 1))
                        # chunk epilogue: reciprocal, normalize, ship
                        r_sb = rp.tile([1, IC], F32, tag="r",
                                       name=f"r{h}_{ci}")
                        nc.vector.reciprocal_approx_fast(r_sb[:], dq[:])
                        rb = rbp.tile([128, IC], F32, tag="rb",
                                      name=f"rb{h}_{ci}")
                        nc.gpsimd.partition_broadcast(rb[:], r_sb[:])
                        o = stg.tile([128, IC], BF16, tag="o",
                                     name=f"o{h}_{ci}")
                        nc.vector.scalar_tensor_tensor(
                            o[:], pv[:], 1.0, rb[:], MULT, MULT)
                        nc.sync.dma_start(
                            a2a_in[g][2 * ci:2 * ci + 2,
                                      hl * 128:(hl + 1) * 128, :]
                            .rearrange("r p s -> p r s"),
                            o[:].rearrange("p (r s) -> p r s", r=2))
                    if hl == GH - 1:
                        nc.gpsimd.collective_compute(
                            "AllToAll", mybir.AluOpType.bypass,
                            replica_groups=[list(range(cfg.R))],
                            ins=[a2a_in[g][:]], outs=[a2a_out[g][:]])

        # ---- phase 4: out_proj on this core's seq block ----
        KB = KO2 - KA
        with (
            tc.tile_pool(name="otf", bufs=1) as otf,
            tc.tile_pool(name="wopA", bufs=4) as wopA,
            tc.tile_pool(name="wopB", bufs=2) as wopB,
            tc.tile_pool(name="obp", bufs=2) as obp,
            tc.tile_pool(name="psB", bufs=8, space="PSUM") as psB,
        ):
            oT_full = otf.tile([128, KO2, SB], BF16)
            for g in range(NG):
                nc.sync.dma_start(
                    oT_full[:, g * (cfg.R * GH):(g + 1) * (cfg.R * GH), :]
                    .rearrange("p (r hl) s -> p r hl s", r=cfg.R),
                    a2a_out[g][:].rearrange("r (hl p) s -> p r hl s", hl=GH))

            def emit_block(ps, wo_t, mi, ko0, nko, start, stop):
                for k in range(nko):
                    ko = ko0 + k
                    nc.tensor.matmul(
                        ps[:], lhsT=oT_full[:, ko, mi * 128:(mi + 1) * 128],
                        rhs=wo_t[:, k, :],
                        start=(start and k == 0),
                        stop=(stop and k == nko - 1))

            def finish_block(ps, nt, mi):
                ob = obp.tile([128, OT], F32, tag="ob", name=f"ob{nt}_{mi}")
                nc.vector.tensor_copy(ob[:], ps[:])
                nc.sync.dma_start(
                    out_d.ap()[mi * 128:(mi + 1) * 128,
                               nt * OT:(nt + 1) * OT],
                    ob[:])

            split_ps = {}
            for nt in range(NSPLIT):
                woA = wopA.tile([128, KA, OT], BF16, tag="woA",
                                name=f"woA{nt}")
                nc.sync.dma_start(woA[:], wo_d.ap()[nt][:, :KA, :])
                for mi in range(SB // 128):
                    ps = psB.tile([128, OT], F32, tag="po",
                                  name=f"po{nt}_{mi}")
                    emit_block(ps, woA, mi, 0, KA, True, False)
                    split_ps[(nt, mi)] = ps
            for nt in range(NSPLIT):
                woB = wopB.tile([128, KB, OT], BF16, tag="woB",
                                name=f"woB{nt}")
                nc.sync.dma_start(woB[:], wo_d.ap()[nt][:, KA:, :])
                for mi in range(SB // 128):
                    ps = split_ps[(nt, mi)]
                    emit_block(ps, woB, mi, KA, KB, False, True)
                    finish_block(ps, nt, mi)
            for nt in range(NSPLIT, NT):
                woA = wopA.tile([128, KA, OT], BF16, tag="woA",
                                name=f"woA{nt}")
                nc.sync.dma_start(woA[:], wo_d.ap()[nt][:, :KA, :])
                woB = wopB.tile([128, KB, OT], BF16, tag="woB",
                                name=f"woB{nt}")
                nc.sync.dma_start(woB[:], wo_d.ap()[nt][:, KA:, :])
                for mi in range(SB // 128):
                    ps = psB.tile([128, OT], F32, tag="po",
                                  name=f"po{nt}_{mi}")
                    emit_block(ps, woA, mi, 0, KA, True, False)
                    emit_block(ps, woB, mi, KA, KB, False, True)
                    finish_block(ps, nt, mi)

    nc.compile()
    return nc


def make_masks(cfg: Cfg) -> np.ndarray:
    ND = cfg.IC // 128
    jj = np.arange(128)[:, None, None]
    rr = np.arange(ND)[None, :, None]
    ii = np.arange(cfg.IC)[None, None, :]
    return (jj + 128 * rr <= ii).astype(NP_BF16)


def shard_inputs(cfg: Cfg, hidden_states, cos, sin, w_qkv, w_out,
                 n_heads, n_kv):
    """Build per-core input maps (host-side shard + bf16 cast + layout)."""
    S, KO, NQ, R = cfg.S, cfg.KO, cfg.NQ, cfg.R
    D = cfg.D
    NCH, CH = S // cfg.CH, cfg.CH
    hid_T = np.ascontiguousarray(hidden_states.reshape(S, D).T)  # [D, S]
    # [NCH, 128, KO, CH]
    hid_l = (hid_T.reshape(KO, 128, NCH, CH).transpose(2, 1, 0, 3)
             .astype(NP_BF16))
    hid_l = np.ascontiguousarray(hid_l)
    NT, OT, KO2 = cfg.NT, cfg.OT, cfg.KO2
    # reorder w_out rows so contraction tile ko2 = g*(R*GH) + r*GH + hl
    # maps to global head r*NQ + g*GH + hl
    NG, GH = cfg.NG, cfg.GH
    w_re = (w_out.reshape(R, NG, GH, 128, cfg.DO).transpose(1, 0, 2, 3, 4)
            .reshape(KO2 * 128, cfg.DO))
    wo_l = (w_re.reshape(KO2, 128, NT, OT).transpose(2, 1, 0, 3)
            .astype(NP_BF16))
    wo_l = np.ascontiguousarray(wo_l)
    cos_T = cos.T.astype(NP_BF16)  # [HD, S]
    sin_T = sin.T
    sinS = np.concatenate([-sin_T[:64], sin_T[64:]], axis=0).astype(NP_BF16)
    masks = make_masks(cfg)

    in_maps = []
    NE = NQ + 2
    NEP = NE // 2
    for c in range(R):
        qs = c * NQ * 128
        # e-tile order: k, v, q0..q5
        wsh = np.concatenate([
            w_qkv[:, n_heads * HD + c * 128: n_heads * HD + (c + 1) * 128],
            w_qkv[:, (n_heads + n_kv) * HD + c * 128:
                  (n_heads + n_kv) * HD + (c + 1) * 128],
            w_qkv[:, qs:qs + NQ * 128],
        ], axis=1)  # [D, NE*128]
        wq_l = (wsh.reshape(KO, 128, NEP, 256).transpose(2, 1, 0, 3)
                .astype(NP_BF16))
        in_maps.append({
            "hid": hid_l, "wq": np.ascontiguousarray(wq_l), "wo": wo_l,
            "cosT": cos_T, "sinT": sinS, "masks": masks,
        })
    return in_maps


_cached = {}


def _get_nc(cfg: Cfg):
    key = tuple(sorted(cfg.__dict__.items()))
    if key not in _cached:
        _cached[key] = build(cfg)
    return _cached[key]


def run(cfg: Cfg, in_maps, **kwargs):
    nc = _get_nc(cfg)
    res = run_bass_kernel_spmd(nc, in_maps, core_ids=list(range(cfg.R)),
                               **kwargs)
    out = np.concatenate([res.results[c]["out"] for c in range(cfg.R)],
                         axis=0)
    return out, res


def kernel(hidden_states, cos, sin, w_qkv, w_out):
    cfg = Cfg()
    hidden_states = np.asarray(hidden_states, dtype=np.float32)
    cos = np.asarray(cos, dtype=np.float32)
    sin = np.asarray(sin, dtype=np.float32)
    w_qkv = np.asarray(w_qkv, dtype=np.float32)
    w_out = np.asarray(w_out, dtype=np.float32)
    in_maps = shard_inputs(cfg, hidden_states, cos, sin, w_qkv, w_out, H, KV)
    out, _ = run(cfg, in_maps)
    return out.reshape(B, S, D).astype(np.float32)


# revision 17
# speedup vs baseline: 1.5297x; 1.2075x over previous
"""DbrxAttention (GQA + RoPE + causal) on 8 Trainium2 NeuronCores.

Tensor-parallel over heads: core c owns q heads [6c, 6c+6) and kv head c.
Per core: QKV projection — k and the 6 q heads in fp8(e3m4)+DoubleRow
(2x PE rate, inputs pre-scaled by 128 host-side; the 2^28 product scale
is folded into the softmax exp), v in bf16; per-chunk RoPE fused into
the projection pipeline. Causal attention in bf16 (scores transposed:
kv on partitions, q on free dim; softmax denominator via ones-column
matmul). 3-way split AllToAll (per head-pair, overlapped with
attention), then each core computes a 256-row sequence block of the
output projection against the full w_out, with the first blocks'
contraction split so work for already-arrived head groups runs while
the last AllToAll is in flight.

kernel(**inputs) takes the full unsharded inputs and returns the full output.
"""

import math

import numpy as np
import ml_dtypes

import concourse.bass as bass
import concourse.mybir as mybir
from concourse import bacc
import concourse.tile as tile
from concourse.bass_utils import run_bass_kernel_spmd
from concourse.masks import make_identity

BF16 = mybir.dt.bfloat16
F32 = mybir.dt.float32
F8 = mybir.dt.float8e4
NP_BF16 = ml_dtypes.bfloat16
NP_F8 = ml_dtypes.float8_e4m3
F8_SCALE = 128.0   # per-operand scale before fp8 cast
F8_CLIP = 224.0    # TRN e4m3 max normal is 240 (inf at 256)

# full-size problem config
B, S, D = 1, 2048, 6144
H, KV, HD = 48, 8, 128
R = 8  # cores


class Cfg:
    def __init__(self, S=2048, KO=48, NQ=6, R=8, DO=6144, IC=512, CH=512,
                 OT=512, GH=2, KA=32, NSPLIT=3):
        self.S = S          # sequence length
        self.KO = KO        # contraction k-tiles for QKV (D = KO*128)
        self.NQ = NQ        # q heads per core
        self.R = R          # cores
        self.DO = DO        # out_proj output dim
        self.IC = IC        # attention i-chunk (free dim per scores matmul)
        self.CH = CH        # QKV s-chunk (pair of CH/2 matmuls)
        self.OT = OT        # out_proj n-chunk
        self.GH = GH        # heads per AllToAll group
        self.KA = KA        # out_proj k-tiles available before last A2A
        self.NSPLIT = NSPLIT  # nt blocks using split-k accumulation
        self.D = KO * 128
        self.SB = S // R    # seq block per core after AllToAll
        self.KO2 = R * NQ   # contraction k-tiles for out_proj (H*HD = KO2*128)
        self.NT = DO // OT
        self.NG = NQ // GH  # number of A2A groups
        assert S % R == 0 and S % IC == 0 and S % CH == 0 and DO % OT == 0
        assert IC % 128 == 0
        assert NQ % GH == 0
        assert KA == self.KO2 - self.R * GH  # kos left for the last group


def build(cfg: Cfg) -> bacc.Bacc:
    S, KO, NQ, IC, CH = cfg.S, cfg.KO, cfg.NQ, cfg.IC, cfg.CH
    NF = NQ + 1            # fp8 e-tiles per core (k + q heads)
    NCH = S // CH
    NIC = S // IC
    ND = IC // 128         # diagonal j-tiles per i-chunk
    NJ = S // 128
    SB, KO2, NT, DO, OT = cfg.SB, cfg.KO2, cfg.NT, cfg.DO, cfg.OT
    GH, NG, KA, NSPLIT = cfg.GH, cfg.NG, cfg.KA, cfg.NSPLIT
    CHH = CH // 2
    KOH = KO // 2          # ko half for split DMAs
    KPH = KO // 4          # fp8 DoubleRow k-pairs per ko half
    softmax_scale = (1.0 / math.sqrt(HD)) / (F8_SCALE ** 4)
    ET_K = 0               # e-tile index of k; q head h is e-tile 1+h
    nc = bacc.Bacc("TRN2", target_bir_lowering=False, debug=False,
                   num_devices=cfg.R)

    hid_d = nc.dram_tensor("hid", [NCH, 128, KO, CH], BF16,
                           kind="ExternalInput")
    hidf_d = nc.dram_tensor("hidf", [NCH, 128, KO, CH], F8,
                            kind="ExternalInput")
    wqv_d = nc.dram_tensor("wqv", [128, KO, 128], BF16,
                           kind="ExternalInput")
    wqf_d = nc.dram_tensor("wqf", [128, NF, KO, 128], F8,
                           kind="ExternalInput")
    wo_d = nc.dram_tensor("wo", [NT, 128, KO2, OT], BF16,
                          kind="ExternalInput")
    cos_d = nc.dram_tensor("cosT", [128, S], BF16, kind="ExternalInput")
    sin_d = nc.dram_tensor("sinT", [128, S], BF16, kind="ExternalInput")
    msk_d = nc.dram_tensor("masks", [128, ND, IC], BF16, kind="ExternalInput")
    out_d = nc.dram_tensor("out", [SB, DO], F32, kind="ExternalOutput")

    with (
        tile.TileContext(nc) as tc,
        tc.tile_pool(name="dram", bufs=1, space="DRAM") as dram,
        tc.tile_pool(name="big", bufs=1) as big,
    ):
        a2a_in = [dram.tile([cfg.R, GH * 128, SB], BF16,
                            name=f"a2a_in{g}") for g in range(NG)]
        a2a_out = [dram.tile([cfg.R, GH * 128, SB], BF16,
                             name=f"a2a_out{g}") for g in range(NG)]
        qkv_sb = big.tile([128, NF, S], BF16)
        ones_sb = big.tile([128, 1], BF16)
        nc.gpsimd.memset(ones_sb[:], 1.0)
        v_nat = big.tile([128, NJ, 128], BF16)
        oT_full = big.tile([128, KO2, SB], BF16)

        with (
            tc.tile_pool(name="psk", bufs=3, space="PSUM") as psk,
            tc.tile_pool(name="psv", bufs=1, space="PSUM") as psv,
            tc.tile_pool(name="psd", bufs=1, space="PSUM") as psd,
        ):
            # ---- phase 1: QKV projection (transposed: [e, s]) with fused
            # per-chunk RoPE and v-transpose. k + q heads via fp8 DoubleRow
            # first, then v in bf16. ----
            with (
                tc.tile_pool(name="ropec", bufs=1) as ropec,
                tc.tile_pool(name="ropep", bufs=2) as ropep,
            ):
                cos_sb = ropec.tile([128, S], BF16)
                sin_sb = ropec.tile([128, S], BF16)
                ident = ropec.tile([128, 128], BF16)

                def rope_chunk(et, ci):
                    sl = slice(ci * CH, (ci + 1) * CH)
                    x = qkv_sb[:, et, sl]
                    rot = ropep.tile([128, CH], BF16, tag="rot",
                                     name=f"rot{et}_{ci}")
                    nc.scalar.copy(rot[0:64, :], x[64:128, :])
                    nc.scalar.copy(rot[64:128, :], x[0:64, :])
                    nc.vector.tensor_mul(rot[:], rot[:], sin_sb[:, sl])
                    nc.vector.tensor_mul(x, x, cos_sb[:, sl])
                    nc.vector.tensor_add(x, x, rot[:])

                # fp8 phase: k + q0..q5, DoubleRow over 24 k-pairs
                with (
                    tc.tile_pool(name="wqp", bufs=1) as wqp,
                    tc.tile_pool(name="hfp", bufs=2) as hfp,
                ):
                    wqf_t = wqp.tile([128, NF, KO, 128], F8)
                    nc.sync.dma_start(wqf_t[:, 0:1], wqf_d.ap()[:, 0:1])
                    for ci in range(NCH):
                        hfa = hfp.tile([128, KOH, CH], F8, tag="hfA",
                                       name=f"hfA{ci}")
                        nc.sync.dma_start(hfa[:],
                                          hidf_d.ap()[ci][:, :KOH, :])
                        hfb = hfp.tile([128, KOH, CH], F8, tag="hfB",
                                       name=f"hfB{ci}")
                        nc.sync.dma_start(hfb[:],
                                          hidf_d.ap()[ci][:, KOH:, :])
                        hf = (hfa, hfb)
                        if ci == 0:
                            for ef in range(1, NF):
                                nc.sync.dma_start(wqf_t[:, ef:ef + 1],
                                                  wqf_d.ap()[:, ef:ef + 1])
                            nc.sync.dma_start(cos_sb[:], cos_d.ap())
                            nc.sync.dma_start(sin_sb[:], sin_d.ap())
                            make_identity(nc, ident[:])
                        for ef in range(NF):
                            ps = psk.tile([128, 1024], F32, tag="ps",
                                          name=f"qk{ef}_{ci}")
                            for kp in range(KO // 2):
                                kh, kl = divmod(kp, KPH)
                                w = wqf_t[:, ef, 2 * kp:2 * kp + 2, :]
                                h_t = hf[kh]
                                st, sp = kp == 0, kp == KO // 2 - 1
                                nc.tensor.matmul(
                                    ps[:, :CHH], lhsT=w,
                                    rhs=h_t[:, 2 * kl:2 * kl + 2, :CHH],
                                    perf_mode=mybir.MatmulPerfMode.DoubleRow,
                                    start=st, stop=sp)
                                nc.tensor.matmul(
                                    ps[:, 512:512 + CHH], lhsT=w,
                                    rhs=h_t[:, 2 * kl:2 * kl + 2, CHH:],
                                    perf_mode=mybir.MatmulPerfMode.DoubleRow,
                                    start=st, stop=sp)
                            nc.vector.tensor_copy(
                                qkv_sb[:, ef, ci * CH:ci * CH + CHH],
                                ps[:, :CHH])
                            nc.vector.tensor_copy(
                                qkv_sb[:, ef, ci * CH + CHH:(ci + 1) * CH],
                                ps[:, 512:512 + CHH])
                            rope_chunk(ef, ci)

                # v phase: bf16, weight-shared matmul pairs, transpose chunks
                with (
                    tc.tile_pool(name="wvp", bufs=1) as wvp,
                    tc.tile_pool(name="hidp", bufs=2) as hidp,
                    tc.tile_pool(name="vsp", bufs=2) as vsp,
                ):
                    wva = wvp.tile([128, KOH, 128], BF16, tag="wvA")
                    nc.sync.dma_start(wva[:], wqv_d.ap()[:, :KOH, :])
                    wvb = wvp.tile([128, KOH, 128], BF16, tag="wvB")
                    nc.sync.dma_start(wvb[:], wqv_d.ap()[:, KOH:, :])
                    wv = (wva, wvb)
                    for ci in range(NCH):
                        ha = hidp.tile([128, KOH, CH], BF16, tag="hidA",
                                       name=f"hidA{ci}")
                        nc.sync.dma_start(ha[:], hid_d.ap()[ci][:, :KOH, :])
                        hb = hidp.tile([128, KOH, CH], BF16, tag="hidB",
                                       name=f"hidB{ci}")
                        nc.sync.dma_start(hb[:], hid_d.ap()[ci][:, KOH:, :])
                        hid_t = (ha, hb)
                        ps = psk.tile([128, 1024], F32, tag="ps",
                                      name=f"vproj{ci}")
                        for ko in range(KO):
                            kh, kl = divmod(ko, KOH)
                            st, sp = ko == 0, ko == KO - 1
                            nc.tensor.matmul(
                                ps[:, :CHH], lhsT=wv[kh][:, kl, :],
                                rhs=hid_t[kh][:, kl, :CHH],
                                start=st, stop=sp)
                            nc.tensor.matmul(
                                ps[:, 512:512 + CHH], lhsT=wv[kh][:, kl, :],
                                rhs=hid_t[kh][:, kl, CHH:],
                                start=st, stop=sp)
                        vstg = vsp.tile([128, CH], BF16, tag="vs",
                                        name=f"vs{ci}")
                        nc.vector.tensor_copy(vstg[:, :CHH], ps[:, :CHH])
                        nc.vector.tensor_copy(vstg[:, CHH:],
                                              ps[:, 512:512 + CHH])
                        for sl in range(CH // 128):
                            st_ = ci * (CH // 128) + sl
                            pt = psv.tile([128, 128], BF16, tag="pv",
                                          name=f"tp{st_}")
                            nc.tensor.transpose(
                                pt[:], vstg[:, sl * 128:(sl + 1) * 128],
                                ident[:])
                            nc.vector.tensor_copy(v_nat[:, st_, :], pt[:])

            # ---- phase 2+3: attention, normalize per chunk, split A2A ----
            with (
                tc.tile_pool(name="attw", bufs=1, side="right") as attw,
                tc.tile_pool(name="pp", bufs=4, side="right") as pp,
                tc.tile_pool(name="rp", bufs=2, side="right") as rp,
                tc.tile_pool(name="rbp", bufs=2, side="right") as rbp,
                tc.tile_pool(name="stg", bufs=3, side="right") as stg,
            ):
                msk_sb = attw.tile([128, ND, IC], BF16)
                nc.sync.dma_start(msk_sb[:], msk_d.ap())
                kT = qkv_sb[:, ET_K, :]
                for h in range(NQ):
                    qT = qkv_sb[:, 1 + h, :]
                    g, hl = divmod(h, GH)
                    for ci in range(NIC):
                        jt_max = (ci + 1) * ND
                        pv = psv.tile([128, IC], F32, tag="pv",
                                      name=f"pv{h}_{ci}")
                        dq = psd.tile([1, IC], F32, tag="dq",
                                      name=f"dq{h}_{ci}")
                        for jp in range(jt_max // 2):
                            sc = psk.tile([128, 1024], F32, tag="ps",
                                          name=f"sc{h}_{ci}_{jp}")
                            for u in range(2):
                                jt = 2 * jp + u
                                nc.tensor.matmul(
                                    sc[:, u * 512:u * 512 + IC],
                                    lhsT=kT[:, jt * 128:(jt + 1) * 128],
                                    rhs=qT[:, ci * IC:(ci + 1) * IC],
                                    start=True, stop=True)
                            p2 = pp.tile([128, 1024], BF16, tag="p",
                                         name=f"p{h}_{ci}_{jp}")
                            nc.scalar.activation(
                                p2[:], sc[:],
                                mybir.ActivationFunctionType.Exp,
                                scale=softmax_scale)
                            for u in range(2):
                                jt = 2 * jp + u
                                pu = p2[:, u * 512:u * 512 + IC]
                                if jt >= ci * ND:
                                    nc.vector.tensor_mul(
                                        pu, pu, msk_sb[:, jt - ci * ND, :])
                                nc.tensor.matmul(
                                    pv[:], lhsT=v_nat[:, jt, :], rhs=pu,
                                    start=(jt == 0), stop=(jt == jt_max - 1))
                                nc.tensor.matmul(
                                    dq[:], lhsT=ones_sb[:, 0:1], rhs=pu,
                                    start=(jt == 0), stop=(jt == jt_max - 1))
                        # chunk epilogue: reciprocal, normalize, ship.
                        # CAST first so the pv PSUM slot frees without
                        # waiting on the reciprocal/broadcast chain.
                        o = stg.tile([128, IC], BF16, tag="o",
                                     name=f"o{h}_{ci}")
                        nc.vector.tensor_copy(o[:], pv[:])
                        r_sb = rp.tile([1, IC], F32, tag="r",
                                       name=f"r{h}_{ci}")
                        nc.vector.reciprocal_approx_fast(r_sb[:], dq[:])
                        rb = rbp.tile([128, IC], F32, tag="rb",
                                      name=f"rb{h}_{ci}")
                        nc.gpsimd.partition_broadcast(rb[:], r_sb[:])
                        nc.vector.tensor_mul(o[:], o[:], rb[:])
                        nc.sync.dma_start(
                            a2a_in[g][2 * ci:2 * ci + 2,
                                      hl * 128:(hl + 1) * 128, :]
                            .rearrange("r p s -> p r s"),
                            o[:].rearrange("p (r s) -> p r s", r=2))
                    if hl == GH - 1:
                        nc.gpsimd.collective_compute(
                            "AllToAll", mybir.AluOpType.bypass,
                            replica_groups=[list(range(cfg.R))],
                            ins=[a2a_in[g][:]], outs=[a2a_out[g][:]])
                        nc.sync.dma_start(
                            oT_full[:, g * (cfg.R * GH):
                                    (g + 1) * (cfg.R * GH), :]
                            .rearrange("p (r hl) s -> p r hl s", r=cfg.R),
                            a2a_out[g][:]
                            .rearrange("r (hl p) s -> p r hl s", hl=GH))

        # ---- phase 4: out_proj on this core's seq block ----
        KB = KO2 - KA
        with (
            tc.tile_pool(name="wopA", bufs=3) as wopA,
            tc.tile_pool(name="wopB", bufs=2) as wopB,
            tc.tile_pool(name="obp", bufs=2) as obp,
            tc.tile_pool(name="psB", bufs=8, space="PSUM") as psB,
        ):
            def emit_block(ps, wo_t, mi, ko0, nko, start, stop):
                for k in range(nko):
                    ko = ko0 + k
                    nc.tensor.matmul(
                        ps[:], lhsT=oT_full[:, ko, mi * 128:(mi + 1) * 128],
                        rhs=wo_t[:, k, :],
                        start=(start and k == 0),
                        stop=(stop and k == nko - 1))

            def finish_block(ps, nt, mi):
                ob = obp.tile([128, OT], F32, tag="ob", name=f"ob{nt}_{mi}")
                nc.vector.tensor_copy(ob[:], ps[:])
                nc.sync.dma_start(
                    out_d.ap()[mi * 128:(mi + 1) * 128,
                               nt * OT:(nt + 1) * OT],
                    ob[:])

            split_ps = {}
            for nt in range(NSPLIT):
                woA = wopA.tile([128, KA, OT], BF16, tag="woA",
                                name=f"woA{nt}")
                nc.sync.dma_start(woA[:], wo_d.ap()[nt][:, :KA, :])
                for mi in range(SB // 128):
                    ps = psB.tile([128, OT], F32, tag="po",
                                  name=f"po{nt}_{mi}")
                    emit_block(ps, woA, mi, 0, KA, True, False)
                    split_ps[(nt, mi)] = ps
            for nt in range(NSPLIT):
                woB = wopB.tile([128, KB, OT], BF16, tag="woB",
                                name=f"woB{nt}")
                nc.sync.dma_start(woB[:], wo_d.ap()[nt][:, KA:, :])
                for mi in range(SB // 128):
                    ps = split_ps[(nt, mi)]
                    emit_block(ps, woB, mi, KA, KB, False, True)
                    finish_block(ps, nt, mi)
            for nt in range(NSPLIT, NT):
                woA = wopA.tile([128, KA, OT], BF16, tag="woA",
                                name=f"woA{nt}")
                nc.sync.dma_start(woA[:], wo_d.ap()[nt][:, :KA, :])
                woB = wopB.tile([128, KB, OT], BF16, tag="woB",
                                name=f"woB{nt}")
                nc.sync.dma_start(woB[:], wo_d.ap()[nt][:, KA:, :])
                for mi in range(SB // 128):
                    ps = psB.tile([128, OT], F32, tag="po",
                                  name=f"po{nt}_{mi}")
                    emit_block(ps, woA, mi, 0, KA, True, False)
                    emit_block(ps, woB, mi, KA, KB, False, True)
                    finish_block(ps, nt, mi)

    nc.compile()
    return nc


def make_masks(cfg: Cfg) -> np.ndarray:
    ND = cfg.IC // 128
    jj = np.arange(128)[:, None, None]
    rr = np.arange(ND)[None, :, None]
    ii = np.arange(cfg.IC)[None, None, :]
    return (jj + 128 * rr <= ii).astype(NP_BF16)


def _to_f8(x):
    return np.ascontiguousarray(
        np.clip(x * F8_SCALE, -F8_CLIP, F8_CLIP)).astype(NP_F8)


def shard_inputs(cfg: Cfg, hidden_states, cos, sin, w_qkv, w_out,
                 n_heads, n_kv):
    """Build per-core input maps (host-side shard + cast + layout)."""
    S, KO, NQ, R = cfg.S, cfg.KO, cfg.NQ, cfg.R
    D = cfg.D
    NCH, CH = S // cfg.CH, cfg.CH
    NF = NQ + 1
    hid_T = np.ascontiguousarray(hidden_states.reshape(S, D).T)  # [D, S]
    # [NCH, 128, KO, CH]
    hid_l = (hid_T.reshape(KO, 128, NCH, CH).transpose(2, 1, 0, 3)
             .astype(NP_BF16))
    hid_l = np.ascontiguousarray(hid_l)
    hidf_l = np.ascontiguousarray(
        _to_f8(hid_T).reshape(KO, 128, NCH, CH).transpose(2, 1, 0, 3))
    NT, OT, KO2 = cfg.NT, cfg.OT, cfg.KO2
    # reorder w_out rows so contraction tile ko2 = g*(R*GH) + r*GH + hl
    # maps to global head r*NQ + g*GH + hl
    NG, GH = cfg.NG, cfg.GH
    w_re = (w_out.reshape(R, NG, GH, 128, cfg.DO).transpose(1, 0, 2, 3, 4)
            .reshape(KO2 * 128, cfg.DO))
    wo_l = (w_re.reshape(KO2, 128, NT, OT).transpose(2, 1, 0, 3)
            .astype(NP_BF16))
    wo_l = np.ascontiguousarray(wo_l)
    cos_T = cos.T.astype(NP_BF16)  # [HD, S]
    sin_T = sin.T
    sinS = np.concatenate([-sin_T[:64], sin_T[64:]], axis=0).astype(NP_BF16)
    masks = make_masks(cfg)

    in_maps = []
    for c in range(R):
        qs = c * NQ * 128
        # fp8 e-tile order: k, q0..q5
        wf = np.concatenate([
            w_qkv[:, n_heads * HD + c * 128: n_heads * HD + (c + 1) * 128],
            w_qkv[:, qs:qs + NQ * 128],
        ], axis=1)  # [D, NF*128]
        wqf_l = (_to_f8(wf).reshape(KO, 128, NF, 128)
                 .transpose(1, 2, 0, 3))  # [128, NF, KO, 128]
        wv = w_qkv[:, (n_heads + n_kv) * HD + c * 128:
                   (n_heads + n_kv) * HD + (c + 1) * 128]  # [D, 128]
        wqv_l = (wv.reshape(KO, 128, 128).transpose(1, 0, 2)
                 .astype(NP_BF16))
        in_maps.append({
            "hid": hid_l, "hidf": hidf_l,
            "wqv": np.ascontiguousarray(wqv_l),
            "wqf": np.ascontiguousarray(wqf_l),
            "wo": wo_l,
            "cosT": cos_T, "sinT": sinS, "masks": masks,
        })
    return in_maps


_cached = {}


def _get_nc(cfg: Cfg):
    key = tuple(sorted(cfg.__dict__.items()))
    if key not in _cached:
        _cached[key] = build(cfg)
    return _cached[key]


def run(cfg: Cfg, in_maps, **kwargs):
    nc = _get_nc(cfg)
    res = run_bass_kernel_spmd(nc, in_maps, core_ids=list(range(cfg.R)),
                               **kwargs)
    out = np.concatenate([res.results[c]["out"] for c in range(cfg.R)],
                         axis=0)
    return out, res


def kernel(hidden_states, cos, sin, w_qkv, w_out):
    cfg = Cfg()
    hidden_states = np.asarray(hidden_states, dtype=np.float32)
    cos = np.asarray(cos, dtype=np.float32)
    sin = np.asarray(sin, dtype=np.float32)
    w_qkv = np.asarray(w_qkv, dtype=np.float32)
    w_out = np.asarray(w_out, dtype=np.float32)
    in_maps = shard_inputs(cfg, hidden_states, cos, sin, w_qkv, w_out, H, KV)
    out, _ = run(cfg, in_maps)
    return out.reshape(B, S, D).astype(np.float32)


# revision 21
# speedup vs baseline: 1.5417x; 1.0079x over previous
"""DbrxAttention (GQA + RoPE + causal) on 8 Trainium2 NeuronCores.

Tensor-parallel over heads: core c owns q heads [6c, 6c+6) and kv head c.
Per core: QKV projection — k and the 6 q heads in fp8(e3m4)+DoubleRow
(2x PE rate, inputs pre-scaled by 128 host-side; the 2^28 product scale
is folded into the softmax exp), v in bf16; per-chunk RoPE fused into
the projection pipeline. Causal attention in bf16 (scores transposed:
kv on partitions, q on free dim; softmax denominator via ones-column
matmul). 3-way split AllToAll (per head-pair, overlapped with
attention), then each core computes a 256-row sequence block of the
output projection against the full w_out, with the first blocks'
contraction split so work for already-arrived head groups runs while
the last AllToAll is in flight.

kernel(**inputs) takes the full unsharded inputs and returns the full output.
"""

import math

import numpy as np
import ml_dtypes

import concourse.bass as bass
import concourse.mybir as mybir
from concourse import bacc
import concourse.tile as tile
from concourse.bass_utils import run_bass_kernel_spmd
from concourse.masks import make_identity

BF16 = mybir.dt.bfloat16
F32 = mybir.dt.float32
F8 = mybir.dt.float8e4
NP_BF16 = ml_dtypes.bfloat16
NP_F8 = ml_dtypes.float8_e4m3
F8_SCALE = 128.0   # per-operand scale before fp8 cast
F8_CLIP = 224.0    # TRN e4m3 max normal is 240 (inf at 256)

# full-size problem config
B, S, D = 1, 2048, 6144
H, KV, HD = 48, 8, 128
R = 8  # cores


class Cfg:
    def __init__(self, S=2048, KO=48, NQ=6, R=8, DO=6144, IC=512, CH=512,
                 OT=512, GH=2, KA=32, NSPLIT=3):
        self.S = S          # sequence length
        self.KO = KO        # contraction k-tiles for QKV (D = KO*128)
        self.NQ = NQ        # q heads per core
        self.R = R          # cores
        self.DO = DO        # out_proj output dim
        self.IC = IC        # attention i-chunk (free dim per scores matmul)
        self.CH = CH        # QKV s-chunk (pair of CH/2 matmuls)
        self.OT = OT        # out_proj n-chunk
        self.GH = GH        # heads per AllToAll group
        self.KA = KA        # out_proj k-tiles available before last A2A
        self.NSPLIT = NSPLIT  # nt blocks using split-k accumulation
        self.D = KO * 128
        self.SB = S // R    # seq block per core after AllToAll
        self.KO2 = R * NQ   # contraction k-tiles for out_proj (H*HD = KO2*128)
        self.NT = DO // OT
        self.NG = NQ // GH  # number of A2A groups
        assert S % R == 0 and S % IC == 0 and S % CH == 0 and DO % OT == 0
        assert IC % 128 == 0
        assert NQ % GH == 0
        assert KA == self.KO2 - self.R * GH  # kos left for the last group


def build(cfg: Cfg) -> bacc.Bacc:
    S, KO, NQ, IC, CH = cfg.S, cfg.KO, cfg.NQ, cfg.IC, cfg.CH
    NF = NQ + 1            # fp8 e-tiles per core (k + q heads)
    NCH = S // CH
    NIC = S // IC
    ND = IC // 128         # diagonal j-tiles per i-chunk
    NJ = S // 128
    SB, KO2, NT, DO, OT = cfg.SB, cfg.KO2, cfg.NT, cfg.DO, cfg.OT
    GH, NG, KA, NSPLIT = cfg.GH, cfg.NG, cfg.KA, cfg.NSPLIT
    CHH = CH // 2
    KOH = KO // 2          # ko half for split DMAs
    KPH = KO // 4          # fp8 DoubleRow k-pairs per ko half
    softmax_scale = (1.0 / math.sqrt(HD)) / (F8_SCALE ** 4)
    ET_K = 0               # e-tile index of k; q head h is e-tile 1+h
    nc = bacc.Bacc("TRN2", target_bir_lowering=False, debug=False,
                   num_devices=cfg.R)

    hid_d = nc.dram_tensor("hid", [NCH, 128, KO, CH], BF16,
                           kind="ExternalInput")
    hidf_d = nc.dram_tensor("hidf", [NCH, 128, KO, CH], F8,
                            kind="ExternalInput")
    wqv_d = nc.dram_tensor("wqv", [128, KO, 128], BF16,
                           kind="ExternalInput")
    wqf_d = nc.dram_tensor("wqf", [128, NF, KO, 128], F8,
                           kind="ExternalInput")
    wo_d = nc.dram_tensor("wo", [NT, 128, KO2, OT], BF16,
                          kind="ExternalInput")
    cos_d = nc.dram_tensor("cosT", [128, S], BF16, kind="ExternalInput")
    sin_d = nc.dram_tensor("sinT", [128, S], BF16, kind="ExternalInput")
    msk_d = nc.dram_tensor("masks", [128, ND, IC], BF16, kind="ExternalInput")
    out_d = nc.dram_tensor("out", [SB, DO], F32, kind="ExternalOutput")

    with (
        tile.TileContext(nc) as tc,
        tc.tile_pool(name="dram", bufs=1, space="DRAM") as dram,
        tc.tile_pool(name="big", bufs=1) as big,
    ):
        a2a_in = [dram.tile([cfg.R, GH * 128, SB], BF16,
                            name=f"a2a_in{g}") for g in range(NG)]
        a2a_out = [dram.tile([cfg.R, GH * 128, SB], BF16,
                             name=f"a2a_out{g}") for g in range(NG)]
        qkv_sb = big.tile([128, NF, S], BF16)
        ones_sb = big.tile([128, 1], BF16)
        nc.gpsimd.memset(ones_sb[:], 1.0)
        v_nat = big.tile([128, NJ, 128], BF16)
        oT_full = big.tile([128, KO2, SB], BF16)

        with (
            tc.tile_pool(name="psk", bufs=3, space="PSUM") as psk,
            tc.tile_pool(name="psv", bufs=1, space="PSUM") as psv,
            tc.tile_pool(name="psd", bufs=1, space="PSUM") as psd,
        ):
            # ---- phase 1: QKV projection (transposed: [e, s]) with fused
            # per-chunk RoPE and v-transpose. k + q heads via fp8 DoubleRow
            # first, then v in bf16. ----
            with (
                tc.tile_pool(name="ropec", bufs=1) as ropec,
                tc.tile_pool(name="ropep", bufs=2) as ropep,
            ):
                cos_sb = ropec.tile([128, S], BF16)
                sin_sb = ropec.tile([128, S], BF16)
                ident = ropec.tile([128, 128], BF16)

                def rope_chunk(et, ci):
                    sl = slice(ci * CH, (ci + 1) * CH)
                    x = qkv_sb[:, et, sl]
                    rot = ropep.tile([128, CH], BF16, tag="rot",
                                     name=f"rot{et}_{ci}")
                    nc.scalar.copy(rot[0:64, :], x[64:128, :])
                    nc.scalar.copy(rot[64:128, :], x[0:64, :])
                    nc.vector.tensor_mul(rot[:], rot[:], sin_sb[:, sl])
                    nc.vector.tensor_mul(x, x, cos_sb[:, sl])
                    nc.vector.tensor_add(x, x, rot[:])

                # fp8 phase: k + q0..q5, DoubleRow over 24 k-pairs
                with (
                    tc.tile_pool(name="wqp", bufs=1) as wqp,
                    tc.tile_pool(name="hfp", bufs=2) as hfp,
                ):
                    wqf_t = wqp.tile([128, NF, KO, 128], F8)
                    nc.sync.dma_start(wqf_t[:, 0:1], wqf_d.ap()[:, 0:1])
                    for ci in range(NCH):
                        hfa = hfp.tile([128, KOH, CH], F8, tag="hfA",
                                       name=f"hfA{ci}")
                        nc.sync.dma_start(hfa[:],
                                          hidf_d.ap()[ci][:, :KOH, :])
                        hfb = hfp.tile([128, KOH, CH], F8, tag="hfB",
                                       name=f"hfB{ci}")
                        nc.sync.dma_start(hfb[:],
                                          hidf_d.ap()[ci][:, KOH:, :])
                        hf = (hfa, hfb)
                        if ci == 0:
                            for ef in range(1, NF):
                                nc.sync.dma_start(wqf_t[:, ef:ef + 1],
                                                  wqf_d.ap()[:, ef:ef + 1])
                            nc.sync.dma_start(cos_sb[:], cos_d.ap())
                            nc.sync.dma_start(sin_sb[:], sin_d.ap())
                            make_identity(nc, ident[:])
                        for ef in range(NF):
                            ps = psk.tile([128, 1024], F32, tag="ps",
                                          name=f"qk{ef}_{ci}")
                            for kp in range(KO // 2):
                                kh, kl = divmod(kp, KPH)
                                w = wqf_t[:, ef, 2 * kp:2 * kp + 2, :]
                                h_t = hf[kh]
                                st, sp = kp == 0, kp == KO // 2 - 1
                                nc.tensor.matmul(
                                    ps[:, :CHH], lhsT=w,
                                    rhs=h_t[:, 2 * kl:2 * kl + 2, :CHH],
                                    perf_mode=mybir.MatmulPerfMode.DoubleRow,
                                    start=st, stop=sp)
                                nc.tensor.matmul(
                                    ps[:, 512:512 + CHH], lhsT=w,
                                    rhs=h_t[:, 2 * kl:2 * kl + 2, CHH:],
                                    perf_mode=mybir.MatmulPerfMode.DoubleRow,
                                    start=st, stop=sp)
                            nc.vector.tensor_copy(
                                qkv_sb[:, ef, ci * CH:ci * CH + CHH],
                                ps[:, :CHH])
                            nc.vector.tensor_copy(
                                qkv_sb[:, ef, ci * CH + CHH:(ci + 1) * CH],
                                ps[:, 512:512 + CHH])
                            rope_chunk(ef, ci)

                # v phase: bf16, weight-shared matmul pairs, transpose
                # chunks. Pools on the right heap side so the DMAs don't
                # wait on fp8-phase address reuse.
                with (
                    tc.tile_pool(name="wvp", bufs=1, side="right") as wvp,
                    tc.tile_pool(name="hidp", bufs=2, side="right") as hidp,
                    tc.tile_pool(name="vsp", bufs=2, side="right") as vsp,
                ):
                    wva = wvp.tile([128, KOH, 128], BF16, tag="wvA")
                    nc.sync.dma_start(wva[:], wqv_d.ap()[:, :KOH, :])
                    wvb = wvp.tile([128, KOH, 128], BF16, tag="wvB")
                    nc.sync.dma_start(wvb[:], wqv_d.ap()[:, KOH:, :])
                    wv = (wva, wvb)
                    for ci in range(NCH):
                        ha = hidp.tile([128, KOH, CH], BF16, tag="hidA",
                                       name=f"hidA{ci}")
                        nc.sync.dma_start(ha[:], hid_d.ap()[ci][:, :KOH, :])
                        hb = hidp.tile([128, KOH, CH], BF16, tag="hidB",
                                       name=f"hidB{ci}")
                        nc.sync.dma_start(hb[:], hid_d.ap()[ci][:, KOH:, :])
                        hid_t = (ha, hb)
                        ps = psk.tile([128, 1024], F32, tag="ps",
                                      name=f"vproj{ci}")
                        for ko in range(KO):
                            kh, kl = divmod(ko, KOH)
                            st, sp = ko == 0, ko == KO - 1
                            nc.tensor.matmul(
                                ps[:, :CHH], lhsT=wv[kh][:, kl, :],
                                rhs=hid_t[kh][:, kl, :CHH],
                                start=st, stop=sp)
                            nc.tensor.matmul(
                                ps[:, 512:512 + CHH], lhsT=wv[kh][:, kl, :],
                                rhs=hid_t[kh][:, kl, CHH:],
                                start=st, stop=sp)
                        vstg = vsp.tile([128, CH], BF16, tag="vs",
                                        name=f"vs{ci}")
                        nc.vector.tensor_copy(vstg[:, :CHH], ps[:, :CHH])
                        nc.vector.tensor_copy(vstg[:, CHH:],
                                              ps[:, 512:512 + CHH])
                        for sl in range(CH // 128):
                            st_ = ci * (CH // 128) + sl
                            pt = psv.tile([128, 128], BF16, tag="pv",
                                          name=f"tp{st_}")
                            nc.tensor.transpose(
                                pt[:], vstg[:, sl * 128:(sl + 1) * 128],
                                ident[:])
                            nc.vector.tensor_copy(v_nat[:, st_, :], pt[:])

            # ---- phase 2+3: attention, normalize per chunk, split A2A ----
            with (
                tc.tile_pool(name="attw", bufs=1, side="right") as attw,
                tc.tile_pool(name="pp", bufs=4, side="right") as pp,
                tc.tile_pool(name="rp", bufs=2, side="right") as rp,
                tc.tile_pool(name="rbp", bufs=2, side="right") as rbp,
                tc.tile_pool(name="stg", bufs=3, side="right") as stg,
            ):
                msk_sb = attw.tile([128, ND, IC], BF16)
                nc.sync.dma_start(msk_sb[:], msk_d.ap())

                def emit_gather(g):
                    nc.sync.dma_start(
                        oT_full[:, g * (cfg.R * GH):
                                (g + 1) * (cfg.R * GH), :]
                        .rearrange("p (r hl) s -> p r hl s", r=cfg.R),
                        a2a_out[g][:]
                        .rearrange("r (hl p) s -> p r hl s", hl=GH))

                kT = qkv_sb[:, ET_K, :]
                for h in range(NQ):
                    qT = qkv_sb[:, 1 + h, :]
                    g, hl = divmod(h, GH)
                    for ci in range(NIC):
                        jt_max = (ci + 1) * ND
                        pv = psv.tile([128, IC], F32, tag="pv",
                                      name=f"pv{h}_{ci}")
                        dq = psd.tile([1, IC], F32, tag="dq",
                                      name=f"dq{h}_{ci}")
                        for jp in range(jt_max // 2):
                            sc = psk.tile([128, 1024], F32, tag="ps",
                                          name=f"sc{h}_{ci}_{jp}")
                            for u in range(2):
                                jt = 2 * jp + u
                                nc.tensor.matmul(
                                    sc[:, u * 512:u * 512 + IC],
                                    lhsT=kT[:, jt * 128:(jt + 1) * 128],
                                    rhs=qT[:, ci * IC:(ci + 1) * IC],
                                    start=True, stop=True)
                            p2 = pp.tile([128, 1024], BF16, tag="p",
                                         name=f"p{h}_{ci}_{jp}")
                            nc.scalar.activation(
                                p2[:], sc[:],
                                mybir.ActivationFunctionType.Exp,
                                scale=softmax_scale)
                            for u in range(2):
                                jt = 2 * jp + u
                                pu = p2[:, u * 512:u * 512 + IC]
                                if jt >= ci * ND:
                                    nc.vector.tensor_mul(
                                        pu, pu, msk_sb[:, jt - ci * ND, :])
                                nc.tensor.matmul(
                                    pv[:], lhsT=v_nat[:, jt, :], rhs=pu,
                                    start=(jt == 0), stop=(jt == jt_max - 1))
                                nc.tensor.matmul(
                                    dq[:], lhsT=ones_sb[:, 0:1], rhs=pu,
                                    start=(jt == 0), stop=(jt == jt_max - 1))
                        # chunk epilogue: reciprocal, normalize, ship.
                        # CAST first so the pv PSUM slot frees without
                        # waiting on the reciprocal/broadcast chain.
                        o = stg.tile([128, IC], BF16, tag="o",
                                     name=f"o{h}_{ci}")
                        nc.vector.tensor_copy(o[:], pv[:])
                        r_sb = rp.tile([1, IC], F32, tag="r",
                                       name=f"r{h}_{ci}")
                        nc.vector.reciprocal_approx_fast(r_sb[:], dq[:])
                        rb = rbp.tile([128, IC], F32, tag="rb",
                                      name=f"rb{h}_{ci}")
                        nc.gpsimd.partition_broadcast(rb[:], r_sb[:])
                        nc.vector.tensor_mul(o[:], o[:], rb[:])
                        nc.sync.dma_start(
                            a2a_in[g][2 * ci:2 * ci + 2,
                                      hl * 128:(hl + 1) * 128, :]
                            .rearrange("r p s -> p r s"),
                            o[:].rearrange("p (r s) -> p r s", r=2))
                    if hl == GH - 1:
                        # gather for the PREVIOUS group first: by now its
                        # collective is done, so the DMA-queue wait is
                        # already satisfied and doesn't block later ships
                        if g > 0:
                            emit_gather(g - 1)
                        nc.gpsimd.collective_compute(
                            "AllToAll", mybir.AluOpType.bypass,
                            replica_groups=[list(range(cfg.R))],
                            ins=[a2a_in[g][:]], outs=[a2a_out[g][:]])

        # ---- phase 4: out_proj on this core's seq block ----
        KB = KO2 - KA
        with (
            tc.tile_pool(name="wopA", bufs=3) as wopA,
            tc.tile_pool(name="wopB", bufs=2) as wopB,
            tc.tile_pool(name="obp", bufs=2) as obp,
            tc.tile_pool(name="psB", bufs=8, space="PSUM") as psB,
        ):
            def emit_block(ps, wo_t, mi, ko0, nko, start, stop):
                for k in range(nko):
                    ko = ko0 + k
                    nc.tensor.matmul(
                        ps[:], lhsT=oT_full[:, ko, mi * 128:(mi + 1) * 128],
                        rhs=wo_t[:, k, :],
                        start=(start and k == 0),
                        stop=(stop and k == nko - 1))

            def finish_block(ps, nt, mi):
                ob = obp.tile([128, OT], F32, tag="ob", name=f"ob{nt}_{mi}")
                nc.vector.tensor_copy(ob[:], ps[:])
                nc.sync.dma_start(
                    out_d.ap()[mi * 128:(mi + 1) * 128,
                               nt * OT:(nt + 1) * OT],
                    ob[:])

            split_ps = {}
            woAs = []
            for nt in range(NSPLIT):
                woA = wopA.tile([128, KA, OT], BF16, tag="woA",
                                name=f"woA{nt}")
                nc.sync.dma_start(woA[:], wo_d.ap()[nt][:, :KA, :])
                woAs.append(woA)
            # last group's gather sits in the DMA queue behind the woA
            # prefetches; by then its collective is done so nothing blocks
            emit_gather(NG - 1)
            for nt in range(NSPLIT):
                for mi in range(SB // 128):
                    ps = psB.tile([128, OT], F32, tag="po",
                                  name=f"po{nt}_{mi}")
                    emit_block(ps, woAs[nt], mi, 0, KA, True, False)
                    split_ps[(nt, mi)] = ps
            for nt in range(NSPLIT):
                woB = wopB.tile([128, KB, OT], BF16, tag="woB",
                                name=f"woB{nt}")
                nc.sync.dma_start(woB[:], wo_d.ap()[nt][:, KA:, :])
                for mi in range(SB // 128):
                    ps = split_ps[(nt, mi)]
                    emit_block(ps, woB, mi, KA, KB, False, True)
                    finish_block(ps, nt, mi)
            for nt in range(NSPLIT, NT):
                woA = wopA.tile([128, KA, OT], BF16, tag="woA",
                                name=f"woA{nt}")
                nc.sync.dma_start(woA[:], wo_d.ap()[nt][:, :KA, :])
                woB = wopB.tile([128, KB, OT], BF16, tag="woB",
                                name=f"woB{nt}")
                nc.sync.dma_start(woB[:], wo_d.ap()[nt][:, KA:, :])
                for mi in range(SB // 128):
                    ps = psB.tile([128, OT], F32, tag="po",
                                  name=f"po{nt}_{mi}")
                    emit_block(ps, woA, mi, 0, KA, True, False)
                    emit_block(ps, woB, mi, KA, KB, False, True)
                    finish_block(ps, nt, mi)

    nc.compile()
    return nc


def make_masks(cfg: Cfg) -> np.ndarray:
    ND = cfg.IC // 128
    jj = np.arange(128)[:, None, None]
    rr = np.arange(ND)[None, :, None]
    ii = np.arange(cfg.IC)[None, None, :]
    return (jj + 128 * rr <= ii).astype(NP_BF16)


def _to_f8(x):
    return np.ascontiguousarray(
        np.clip(x * F8_SCALE, -F8_CLIP, F8_CLIP)).astype(NP_F8)


def shard_inputs(cfg: Cfg, hidden_states, cos, sin, w_qkv, w_out,
                 n_heads, n_kv):
    """Build per-core input maps (host-side shard + cast + layout)."""
    S, KO, NQ, R = cfg.S, cfg.KO, cfg.NQ, cfg.R
    D = cfg.D
    NCH, CH = S // cfg.CH, cfg.CH
    NF = NQ + 1
    hid_T = np.ascontiguousarray(hidden_states.reshape(S, D).T)  # [D, S]
    # [NCH, 128, KO, CH]
    hid_l = (hid_T.reshape(KO, 128, NCH, CH).transpose(2, 1, 0, 3)
             .astype(NP_BF16))
    hid_l = np.ascontiguousarray(hid_l)
    hidf_l = np.ascontiguousarray(
        _to_f8(hid_T).reshape(KO, 128, NCH, CH).transpose(2, 1, 0, 3))
    NT, OT, KO2 = cfg.NT, cfg.OT, cfg.KO2
    # reorder w_out rows so contraction tile ko2 = g*(R*GH) + r*GH + hl
    # maps to global head r*NQ + g*GH + hl
    NG, GH = cfg.NG, cfg.GH
    w_re = (w_out.reshape(R, NG, GH, 128, cfg.DO).transpose(1, 0, 2, 3, 4)
            .reshape(KO2 * 128, cfg.DO))
    wo_l = (w_re.reshape(KO2, 128, NT, OT).transpose(2, 1, 0, 3)
            .astype(NP_BF16))
    wo_l = np.ascontiguousarray(wo_l)
    cos_T = cos.T.astype(NP_BF16)  # [HD, S]
    sin_T = sin.T
    sinS = np.concatenate([-sin_T[:64], sin_T[64:]], axis=0).astype(NP_BF16)
    masks = make_masks(cfg)

    in_maps = []
    for c in range(R):
        qs = c * NQ * 128
        # fp8 e-tile order: k, q0..q5
        wf = np.concatenate([
            w_qkv[:, n_heads * HD + c * 128: n_heads * HD + (c + 1) * 128],
            w_qkv[:, qs:qs + NQ * 128],
        ], axis=1)  # [D, NF*128]
        wqf_l = (_to_f8(wf).reshape(KO, 128, NF, 128)
                 .transpose(1, 2, 0, 3))  # [128, NF, KO, 128]
        wv = w_qkv[:, (n_heads + n_kv) * HD + c * 128:
                   (n_heads + n_kv) * HD + (c + 1) * 128]  # [D, 128]
        wqv_l = (wv.reshape(KO, 128, 128).transpose(1, 0, 2)
                 .astype(NP_BF16))
        in_maps.append({
            "hid": hid_l, "hidf": hidf_l,
            "wqv": np.ascontiguousarray(wqv_l),
            "wqf": np.ascontiguousarray(wqf_l),
            "wo": wo_l,
            "cosT": cos_T, "sinT": sinS, "masks": masks,
        })
    return in_maps


_cached = {}


def _get_nc(cfg: Cfg):
    key = tuple(sorted(cfg.__dict__.items()))
    if key not in _cached:
        _cached[key] = build(cfg)
    return _cached[key]


def run(cfg: Cfg, in_maps, **kwargs):
    nc = _get_nc(cfg)
    res = run_bass_kernel_spmd(nc, in_maps, core_ids=list(range(cfg.R)),
                               **kwargs)
    out = np.concatenate([res.results[c]["out"] for c in range(cfg.R)],
                         axis=0)
    return out, res


def kernel(hidden_states, cos, sin, w_qkv, w_out):
    cfg = Cfg()
    hidden_states = np.asarray(hidden_states, dtype=np.float32)
    cos = np.asarray(cos, dtype=np.float32)
    sin = np.asarray(sin, dtype=np.float32)
    w_qkv = np.asarray(w_qkv, dtype=np.float32)
    w_out = np.asarray(w_out, dtype=np.float32)
    in_maps = shard_inputs(cfg, hidden_states, cos, sin, w_qkv, w_out, H, KV)
    out, _ = run(cfg, in_maps)
    return out.reshape(B, S, D).astype(np.float32)
